# revision 22
# baseline (speedup 1.0000x reference)
"""BinaryConv (BN + sign-binarize + 3x3 binary conv) on 8 Trainium2 NeuronCores.

Strategy (data-parallel over batch, per sharding hint):
  - Each of the 8 cores gets 4 of the 32 images; weights/gamma/beta replicated.
  - Sync-BN exact: per-core partial stats (mean, meansq per channel) are
    summed across all 8 cores (2KB payload).
  - Per-core partial stats (bn_stats on DVE, consumed in DMA-arrival order)
    are AllGathered via collective_compute (2KB payload, single ring pass)
    and tree-reduced locally on DVE. The stats bounce DMA fires only after
    the x load drains (tiny SWDGE DMAs concurrent with bulk HBM traffic
    stall all 16 SDMA engines on write-receipt round-trips), and the 2.25MB
    W stream is gated behind the bounce so the CC doorbell is never starved.
    (A remote_dma XOR-allreduce alternative, sim-validated but failing on
    this runtime, is kept behind USE_REMOTE_RDH.)
  - W is pre-permuted on host to [ci_local, dy, dx, ci_half, o] so a single
    Sign activation emits the binarized weights directly in the fp8 DoubleRow
    lhsT layout — no PE transposes, no DVE copies.
  - Binarize via ScalarE Sign(gamma*x + (beta*std - mean*gamma)) into a
    zero-padded per-image layout (34-wide rows, both ci-halves stacked) fp8.
    std is computed on DVE (Heron iterations) so ScalarE never switches
    activation tables after its initial load.
  - 3x3 conv = 9 shifted DoubleRow fp8 matmuls (contraction 256 in one pass)
    accumulated in PSUM; +/-1 inputs with fp32 PSUM accumulation are exact.
    o-block-outer loop halves output-completion granularity; per-chunk output
    DMAs alternate the two HWDGE queues.
  - A warm burst of matmuls gated on the exchange completion un-throttles
    the PE clock (HAM) during stat-math + first binarize, so the conv starts
    warm.
"""

import numpy as np

import concourse.tile as tile
from concourse import bacc, library_config, mybir
from concourse.bass_utils import run_bass_kernel_spmd

F32 = mybir.dt.float32
BF16 = mybir.dt.bfloat16
FP8 = mybir.dt.float8e4

N_CORES = 8
N = 32            # full batch
NLOC = N // N_CORES  # images per core
C = 256           # channels (in == out)
HW = 32           # spatial
CB = C // 128     # ci partition blocks
OB = C // 128     # o partition blocks
EPS = 1e-5

USE_DUMMY_CC = False  # A/B flag: early garbage-payload AllGather
USE_REMOTE_RDH = False  # stats allreduce via remote_dma XOR exchange (no CC)

PADW = HW + 2     # padded row width
IMG_PAD = 1160    # per-image padded buffer (>= 34*34 + 2 margin, mult of 8)
# output row-chunks (r0, r1): each chunk's matmul free dim = (r1-r0)*34 <= 512
CHUNKS = [(0, 11), (11, 22), (22, 32)]
TAPS = [(dy, dx) for dy in range(3) for dx in range(3)]
NPIX = HW * HW
WARM_MMS = 6


def _build_body(ctx, nc, tc, x_d, g_d, be_d, w_d, y_d,
                dcc_in, dcc_out, cc_in, cc_out):
    # ---------------- pools ----------------
    const = ctx.enter_context(tc.tile_pool(name="const", bufs=1))
    xin_p = ctx.enter_context(tc.tile_pool(name="xin", bufs=1))
    wpool = ctx.enter_context(tc.tile_pool(name="wpool", bufs=1))
    apool = ctx.enter_context(tc.tile_pool(name="apool", bufs=1))
    stat_p = ctx.enter_context(tc.tile_pool(name="stat", bufs=1))
    out_p = ctx.enter_context(tc.tile_pool(name="outp", bufs=1))
    ps_acc = ctx.enter_context(tc.tile_pool(name="psacc", bufs=1, space="PSUM"))
    ps_warm = ctx.enter_context(tc.tile_pool(name="pswarm", bufs=1, space="PSUM"))

    # The remote-DMA desc-gen ops live in GPSIMD ucode library 10; load it
    # up front (the Q7 executes unimplemented opcodes as garbage otherwise —
    # the simulator does not model library state, so only HW catches this).
    # memset/dma_start/wait/trigger are dispatcher-native and unaffected.
    if USE_REMOTE_RDH:
        nc.gpsimd.load_library(library_config.remote_dma)

    # ---------------- dummy collective: skew/init-barrier absorber ----------
    # Doorbell rings at ~1us with NO bounce DMA (the gathered payload is
    # garbage and never read): the once-per-launch CC init barrier + launch
    # skew are absorbed during the x-load window. A bounce here would be
    # poison — tiny SWDGE DMAs concurrent with bulk HBM traffic stall all 16
    # SDMA engines on write-receipt round-trips (~7us fabric stall measured).
    if USE_DUMMY_CC:
        nc.gpsimd.collective_compute(
            "AllGather",
            mybir.AluOpType.bypass,
            replica_groups=[list(range(N_CORES))],
            ins=[dcc_in.ap().opt()],
            outs=[dcc_out.ap().opt()],
        )

    # ---------------- load x (stats-critical) ----------------
    # Two HWDGE issue queues (Sync: block 0, Scalar: block 1) stream in
    # parallel; per-queue FIFO keeps W behind the last x chunk.
    xin = []
    last_x_dma = [None, None]
    for b in range(CB):
        xb = xin_p.tile([128, NLOC, HW, HW], F32, name=f"xin{b}", tag=f"xin{b}")
        eng = nc.sync if b == 0 else nc.scalar
        for i in range(NLOC):
            last_x_dma[b] = eng.dma_start(
                out=xb[:, i], in_=x_d[i, 128 * b : 128 * (b + 1), :, :]
            )
        xin.append(xb)

    # gamma/beta (tiny, off critical path)
    gam = stat_p.tile([128, CB], F32, name="gam")
    bet = stat_p.tile([128, CB], F32, name="bet")
    for b in range(CB):
        nc.sync.dma_start(out=gam[:, b : b + 1], in_=g_d[128 * b : 128 * (b + 1), :])
        nc.sync.dma_start(out=bet[:, b : b + 1], in_=be_d[128 * b : 128 * (b + 1), :])

    # ---------------- zero only the padding of the activation buffers ------
    apad = [None] * NLOC
    for i in range(NLOC):
        ap = apool.tile([128, CB, IMG_PAD], FP8, name=f"apad{i}",
                        tag=f"apad{i}")
        nc.gpsimd.memset(ap[:, :, 0:35], 0.0)
        gaps = ap[:, :, 67 : 67 + 34 * HW].rearrange(
            "p b (h w) -> p b h w", w=PADW
        )[:, :, :, 0:2]
        nc.gpsimd.memset(gaps, 0.0)
        nc.gpsimd.memset(ap[:, :, 35 + 34 * HW - 2 : IMG_PAD], 0.0)
        apad[i] = ap

    # warm-burst junk operands (any bf16 data works; 1.0 keeps it NaN-free)
    junk = const.tile([128, 512], BF16, name="junk")
    nc.vector.memset(junk[:], 1.0)

    # ---------------- local BN partial stats (DVE bn_stats) ----------------
    # 512-elem records (the bn_stats hardware cap), consumed in DMA arrival
    # order so DVE tracks the HBM-bound x load.
    stats_rec = [
        stat_p.tile([128, 2 * NLOC, 6], F32, name=f"rec{b}", tag=f"rec{b}")
        for b in range(CB)
    ]
    # block 0's records first so its aggregate+pack overlaps block 1's tail
    # (DVE consumes in DMA-arrival order either way; the two queues stream
    # their blocks in parallel).
    arbuf = stat_p.tile([128, 2 * CB], F32, name="arbuf")
    tmp1 = stat_p.tile([128, 1], F32, name="tmp1")

    def _pack(b):
        mv = stat_p.tile([128, 2], F32, name=f"mv{b}", tag=f"mv{b}")
        nc.vector.bn_aggr(out=mv[:], in_=stats_rec[b][:])
        nc.vector.tensor_copy(out=arbuf[:, 2 * b : 2 * b + 1], in_=mv[:, 0:1])
        nc.vector.tensor_mul(tmp1[:], mv[:, 0:1], mv[:, 0:1])
        nc.vector.tensor_add(arbuf[:, 2 * b + 1 : 2 * b + 2], mv[:, 1:2],
                             tmp1[:])

    for i in range(NLOC):
        for b in range(CB):
            for h in range(2):
                nc.vector.bn_stats(
                    out=stats_rec[b][:, 2 * i + h, :],
                    in_=xin[b][:, i, 16 * h : 16 * (h + 1), :].rearrange(
                        "p h w -> p (h w)"
                    ),
                )
        if i == NLOC - 1:
            _pack(0)
    _pack(1)

    # ---------------- global reduction of the partial stats ----------------
    if USE_REMOTE_RDH:
        # 3-stage XOR recursive-doubling allreduce over remote_dma (SBUF to
        # SBUF across cores, semaphore-synchronized) — bypasses the collective
        # framework's barrier/ncfw latency entirely. Relative destinations
        # (own tpb XOR bit) make rank->physical mapping irrelevant: any
        # bijection still pairs all 8 cores. All cores produce bit-identical
        # sums (commutative same-bracketing adds).
        racc = [arbuf] + [
            stat_p.tile([128, 2 * CB], F32, name=f"racc{s}") for s in (1, 2, 3)
        ]
        rrb = [
            stat_p.tile([128, 2 * CB], F32, name=f"rrb{s}") for s in range(3)
        ]
        rsem = [nc.alloc_semaphore(f"rdh_rsem{s}") for s in range(3)]
        rl_sem = nc.alloc_semaphore("rdh_lsem")
        dve_done = nc.alloc_semaphore("rdh_dve_done")
        with tc.tile_critical(name="rdh"):
            for s, bit in enumerate((1, 2, 4)):
                rdests = [None] * 8
                rdests[4 if bit == 4 else 0] = (0, bit)
                if s > 0:
                    nc.gpsimd.wait_ge(dve_done, s)
                nc.gpsimd.remote_dma_broadcast(
                    out_ap=rrb[s][:], in_ap=racc[s][:],
                    remote_sem=rsem[s], local_sem=rl_sem, rdests=rdests,
                )
                nc.gpsimd.trigger_dma(count=None)
                nc.vector.wait_ge(rsem[s], 2)
                ai = nc.vector.tensor_add(racc[s + 1][:], racc[s][:],
                                          rrb[s][:])
                if s < 2:
                    ai.then_inc(dve_done, 1)
        gs = racc[3]
        w_gate = [last_x_dma[0].ins, last_x_dma[1].ins]
    else:
        bounce_dma = nc.sync.dma_start(out=cc_in[:, :], in_=arbuf[:])
        nc.gpsimd.collective_compute(
            "AllGather",
            mybir.AluOpType.bypass,
            replica_groups=[list(range(N_CORES))],
            ins=[cc_in.ap().opt()],
            outs=[cc_out.ap().opt()],
        )
        w_gate = [bounce_dma.ins, bounce_dma.ins]

    # ---------------- weight load + sign (inside the exchange window) ----
    # Host pre-permuted W: [128 ci_local, 9 taps, 2 ci_half, 256 o] f32.
    # In CC mode the 2.25MB W stream must wait for the tiny HBM bounce (it
    # would starve it in the SDMA packet round-robin); the RDH exchange is
    # SBUF-to-SBUF so W can start right behind the x load. W has ~25us of
    # slack before the conv needs it.
    wraw = wpool.tile([128, len(TAPS), CB, C], F32, name="wraw")
    wT = wpool.tile([128, len(TAPS), CB, C], FP8, name="wT")
    half = len(TAPS) * CB * C // 2
    wv = wraw[:].rearrange("p t h o -> p (t h o)")
    for q, eng in enumerate((nc.sync, nc.scalar)):
        wdma = eng.dma_start(
            out=wv[:, q * half : (q + 1) * half],
            in_=w_d[:, q * half : (q + 1) * half],
        )
        tile.add_dep_helper(
            wdma.ins, w_gate[q], sync=True, reason="W after load/bounce"
        )
        nc.scalar.activation(
            out=wT[:].rearrange("p t h o -> p (t h o)")[:, q * half : (q + 1) * half],
            in_=wv[:, q * half : (q + 1) * half],
            func=mybir.ActivationFunctionType.Sign,
        )

    if not USE_REMOTE_RDH:
        # readback all 8 ranks' partials, split across the two HWDGE queues
        gsall = stat_p.tile([128, N_CORES, 2 * CB], F32, name="gsall")
        hr = N_CORES // 2
        nc.sync.dma_start(
            out=gsall[:, 0:hr, :],
            in_=cc_out[0:hr, :, :].rearrange("k p s -> p k s"),
        )
        nc.scalar.dma_start(
            out=gsall[:, hr:, :],
            in_=cc_out[hr:, :, :].rearrange("k p s -> p k s"),
        )
        gs4 = stat_p.tile([128, 4, 2 * CB], F32, name="gs4")
        nc.vector.tensor_add(gs4[:], gsall[:, 0:4, :], gsall[:, 4:8, :])
        gs2 = stat_p.tile([128, 2, 2 * CB], F32, name="gs2")
        nc.vector.tensor_add(gs2[:], gs4[:, 0:2, :], gs4[:, 2:4, :])
        gs = stat_p.tile([128, 2 * CB], F32, name="gs")
        nc.vector.tensor_add(gs[:], gs2[:, 0, :], gs2[:, 1, :])

    ssx = gs[:].rearrange("p (b s) -> p b s", s=2)[:, :, 0]   # [128, CB]
    ssxx = gs[:].rearrange("p (b s) -> p b s", s=2)[:, :, 1]
    inv = 1.0 / N_CORES  # per-rank means/meansqs -> global average
    mean_t = stat_p.tile([128, CB], F32, name="mean_t")
    mean_ins = nc.vector.tensor_scalar_mul(out=mean_t[:], in0=ssx, scalar1=inv)

    # ---------------- PE warm burst (gated on exchange completion) ----------
    # ~3.4us of matmuls un-throttle the HAM clock gate while DVE finishes the
    # stat math and ScalarE starts binarizing; the conv then opens at 2.4GHz.
    for k in range(WARM_MMS):
        pw = ps_warm.tile([128, 512], F32, name="pw", tag="pw", bufs=1)
        wm = nc.tensor.matmul(pw[:], junk[:, 0:128], junk[:], start=True,
                              stop=True)
        if k == 0:
            tile.add_dep_helper(
                wm.ins, mean_ins.ins, sync=True,
                reason="warm burst after stats exchange",
            )
    msqr = stat_p.tile([128, CB], F32, name="msqr")
    nc.vector.tensor_mul(msqr[:], mean_t[:], mean_t[:])
    var_t = stat_p.tile([128, CB], F32, name="var_t")
    nc.vector.scalar_tensor_tensor(
        out=var_t[:], in0=ssxx, scalar=inv, in1=msqr[:],
        op0=mybir.AluOpType.mult, op1=mybir.AluOpType.subtract,
    )
    # std = sqrt(var + EPS) on DVE via 2 Heron iterations (no ScalarE table
    # switch). x is unit-normal per spec, so var is within a few percent of
    # 1.0 (32K samples/channel): seed s0=1 -> rel err ~4e-8 after 2 iters.
    vpe = stat_p.tile([128, CB], F32, name="vpe")
    nc.vector.tensor_scalar_add(out=vpe[:], in0=var_t[:], scalar1=EPS)
    s1 = stat_p.tile([128, CB], F32, name="s1")
    nc.vector.tensor_scalar(
        out=s1[:], in0=vpe[:], scalar1=0.5, scalar2=0.5,
        op0=mybir.AluOpType.mult, op1=mybir.AluOpType.add,
    )                                                     # s1 = (1+v)/2
    rc = stat_p.tile([128, CB], F32, name="rc")
    nc.vector.reciprocal(out=rc[:], in_=s1[:])
    nc.vector.tensor_mul(rc[:], rc[:], vpe[:])            # v/s1
    std_t = stat_p.tile([128, CB], F32, name="std_t")
    nc.vector.tensor_add(std_t[:], s1[:], rc[:])
    nc.vector.tensor_scalar_mul(out=std_t[:], in0=std_t[:], scalar1=0.5)
    # shift = beta*std - mean*gamma
    sh_t = stat_p.tile([128, CB], F32, name="sh_t")
    nmg = stat_p.tile([128, CB], F32, name="nmg")
    nc.vector.tensor_mul(nmg[:], mean_t[:], gam[:])
    nc.vector.tensor_mul(sh_t[:], bet[:], std_t[:])
    nc.vector.tensor_sub(sh_t[:], sh_t[:], nmg[:])
    scale_t = [gam[:, b : b + 1] for b in range(CB)]
    shift_t = [sh_t[:, b : b + 1] for b in range(CB)]

    # ---------------- binarize into padded layout (fp8, DoubleRow pairs) ----
    for i in range(NLOC):
        for h in range(2):
            for b in range(CB):
                interior = apad[i][:, b, 35 : 35 + 34 * HW].rearrange(
                    "p (h w) -> p h w", w=PADW
                )[:, 16 * h : 16 * (h + 1), 0:HW]
                nc.scalar.activation(
                    out=interior,
                    in_=xin[b][:, i, 16 * h : 16 * (h + 1), :],
                    func=mybir.ActivationFunctionType.Sign,
                    scale=scale_t[b],
                    bias=shift_t[b],
                )

    # ---------------- conv: 9 shifted DoubleRow matmuls, PSUM accumulate ----
    # o-outer: each (img, o) completes after 27 matmuls; its chunk copies and
    # output DMAs overlap the next (img, o) group's matmuls.
    dma_engs = (nc.sync, nc.scalar)
    nd = 0
    for i in range(NLOC):
        for o in range(OB):
            psum = [
                ps_acc.tile([128, (r1 - r0) * PADW], F32, name=f"acc{o}_{ci}",
                            tag=f"acc{o}_{ci}", bufs=1)
                for ci, (r0, r1) in enumerate(CHUNKS)
            ]
            for t in range(len(TAPS)):
                dy, dx = TAPS[t]
                toff = dy * PADW + dx
                lhsT = wT[:, t, :, 128 * o : 128 * (o + 1)]
                for ci, (r0, r1) in enumerate(CHUNKS):
                    ncols = (r1 - r0) * PADW
                    off = r0 * PADW + toff
                    nc.tensor.matmul(
                        psum[ci][:],
                        lhsT,
                        apad[i][:, :, off : off + ncols],
                        start=(t == 0),
                        stop=(t == len(TAPS) - 1),
                        perf_mode=mybir.MatmulPerfMode.DoubleRow,
                    )
            osb = out_p.tile([128, HW, HW], F32, name=f"osb{o}", tag=f"osb{o}",
                             bufs=2)
            for ci, (r0, r1) in enumerate(CHUNKS):
                nc.vector.tensor_copy(
                    out=osb[:, r0:r1, :],
                    in_=psum[ci][:].rearrange("p (r c) -> p r c", c=PADW)[
                        :, :, 0:HW
                    ],
                )
                dma_engs[nd % 2].dma_start(
                    out=y_d[i, 128 * o : 128 * (o + 1), r0:r1, :],
                    in_=osb[:, r0:r1, :],
                )
                nd += 1


_CACHE: dict = {}


def _build():
    if "nc" in _CACHE:
        return _CACHE["nc"]
    nc = bacc.Bacc(
        "TRN2", target_bir_lowering=False, debug=False, num_devices=N_CORES
    )
    x_d = nc.dram_tensor("x", [NLOC, C, HW, HW], F32, kind="ExternalInput")
    g_d = nc.dram_tensor("gamma", [C, 1], F32, kind="ExternalInput")
    be_d = nc.dram_tensor("beta", [C, 1], F32, kind="ExternalInput")
    w_d = nc.dram_tensor("w", [128, len(TAPS) * CB * C], F32,
                         kind="ExternalInput")
    y_d = nc.dram_tensor("y", [NLOC, C, HW, HW], F32, kind="ExternalOutput")
    dcc_in = nc.dram_tensor("dcc_in", [128, 1], F32)
    dcc_out = nc.dram_tensor(
        "dcc_out", [N_CORES, 128, 1], F32, addr_space="Shared"
    )
    cc_in = nc.dram_tensor("cc_in", [128, 2 * CB], F32)
    cc_out = nc.dram_tensor(
        "cc_out", [N_CORES, 128, 2 * CB], F32, addr_space="Shared"
    )

    from contextlib import ExitStack

    with tile.TileContext(nc) as tc, ExitStack() as ctx:
        _build_body(ctx, nc, tc, x_d, g_d, be_d, w_d, y_d,
                    dcc_in, dcc_out, cc_in, cc_out)
    nc.compile()
    _CACHE["nc"] = nc
    return nc


def _prep_w(W: np.ndarray) -> np.ndarray:
    # [o, ci, dy, dx] -> [ci_local, dy, dx, ci_half, o], flattened for the
    # device-side Sign into the fp8 DoubleRow lhsT layout.
    V = W.reshape(C, CB, 128, 3, 3).transpose(2, 3, 4, 1, 0)
    return np.ascontiguousarray(V.reshape(128, len(TAPS) * CB * C))


def kernel(x, gamma, beta, W):
    x = np.ascontiguousarray(np.asarray(x, dtype=np.float32))
    gamma = np.ascontiguousarray(np.asarray(gamma, dtype=np.float32)).reshape(C, 1)
    beta = np.ascontiguousarray(np.asarray(beta, dtype=np.float32)).reshape(C, 1)
    W = np.ascontiguousarray(np.asarray(W, dtype=np.float32))
    Wp = _prep_w(W)
    nc = _build()
    in_maps = [
        {
            "x": x[NLOC * k : NLOC * (k + 1)],
            "gamma": gamma,
            "beta": beta,
            "w": Wp,
        }
        for k in range(N_CORES)
    ]
    res = run_bass_kernel_spmd(nc, in_maps, core_ids=list(range(N_CORES)))
    return np.concatenate(
        [res.results[k]["y"] for k in range(N_CORES)], axis=0
    )
